# revision 55
# baseline (speedup 1.0000x reference)
"""AttentionPairBias Trainium2 Bass kernel, 8-way query-sharded.

Per core (N=768, D=768, H=16, HD=48, ZD=128): core d owns query rows
[d*96,(d+1)*96). z AND z^2 arrive host-cast to fp8e4m3 (the fp8 z^2
stream replaces all on-device squaring; LN statistics still reduce on
device), host-transposed to [kc, qb, c=128, q=24, k=128] so every DMA
line is contiguous and the ZD contraction dim is on partitions. Per
z tile the PE does the whole LN reduction with per-q-column
stationaries: u8[k,q,17] = zt_q^T @ W2' (mean rides as a ones/128
column, mixed fp8xbf16 matmul) and ssq[k,q] = zsq_q^T @ ones. Both
rstd paths (z-bias and LN(s)) use a table-free DVE Newton step, so Exp
is the only ACT table ever loaded (preloaded at kernel start).
LayerNorm folds: W2' = z_norm_w*z_w - ones*colsum/128; the k-side
projection bias is dropped (softmax shift invariance); LN(s)
weight/bias, q_b and the 1/sqrt(HD) scale fold into projection weights
host-side; s ships as bf16; each projection bias rides as row 0 of an
extra 128-row block of its weight tensor; v_b is folded into the o
accumulator via two rank-1 PSUM matmuls (softmax rows sum to 1).

The pair bias is accumulated into the scores PSUM by identity matmuls
on the PE (u_kc is materialized [k,h,q] so the moving operand is
contiguous per head), and exp reads the PSUM bank directly on ACT.
DMA issue is balanced over the SP HWDGE and Pool SWDGE queues (a DMA
occupies its issuing engine for the whole transfer in this machine
model; ACT carries only s_full since its exec-queue depth of 0 would
stall activations). The main loop is software-pipelined: k projection
runs two chunks ahead, the first half of chunk t+1's z projection is
emitted before chunk t's scores so its DVE stats overlap exp/PV, and
the lead-in interleaves per-(d,t) snT transpose blocks with the k/z
pipelines so the PE never queues behind a not-yet-normalized token
chunk. The tail pipelines normalize/gate/transpose/project by head
halves and splits the output DMA across the SP and ACT queues.
"""

from contextlib import ExitStack

import numpy as np
import ml_dtypes

import concourse.bass as bass
import concourse.mybir as mybir
from concourse.tile import TileContext
from concourse.vector_clock import ScopedClock
from concourse.masks import make_identity

F32 = mybir.dt.float32
BF16 = mybir.dt.bfloat16
FP8 = mybir.dt.float8e4
AF = mybir.ActivationFunctionType
ALU = mybir.AluOpType

N_CORES = 8
EPS = 1e-5
EXP_SHIFT = 3.0


def _patch_tile_drain():
    """walrus in this container caps sync waits per CTRL instruction; spread
    the TileContext tail-drain waits across single-wait SP nops."""
    if getattr(TileContext, "_drain_patched", False):
        return

    def _drain_and_barrier(self, tick_clock, wait_clock):
        nc = self.nc
        probe = nc.sync.nop(nofuse=True, hint="tail_wait_probe")
        wait_clock.add_sem_waits(probe.ins, ScopedClock({None: tick_clock.global_clock}))
        si = probe.ins.sync_info
        waits = list(si.on_wait or []) if si else []
        if len(waits) > 1:
            si.on_wait = waits[:1]
            for w in waits[1:]:
                n2 = nc.sync.nop(nofuse=True, hint="tail_wait_split")
                n2.ins.sync_info = mybir.SyncInfo(on_wait=[w], on_update=[])
        nc.sync.drain()
        nc.all_engine_barrier()
        assert self.sems is not None
        popped = nc._tile_sem_poison_stack.pop()
        assert popped is self._sem_poison
        nc.clear_and_free_semaphores(list(self.sems.allocated().values()))
        nc.all_engine_barrier()

    TileContext._drain_and_barrier = _drain_and_barrier
    TileContext._drain_patched = True


def _split_excess_waits(nc, cap=1):
    """walrus in this container rejects instructions with more than ~2 sync
    waits; move the excess onto same-engine NOPs placed just before."""
    ctr = [0]

    def mk_nop(engine, waits):
        ctr[0] += 1
        nop = mybir.InstNoOp(name=f"I-waitsplit-{ctr[0]}", ins=[], outs=[])
        nop.engine = engine
        nop.sync_info = mybir.SyncInfo(on_wait=waits, on_update=[])
        return nop

    for f in nc.m.functions:
        for bb in f.blocks:
            out, changed = [], False
            for inst in bb.instructions:
                si = inst.sync_info
                waits = list(si.on_wait) if si and si.on_wait else []
                if len(waits) > cap:
                    excess = waits[:-cap]
                    for i in range(0, len(excess), cap):
                        out.append(mk_nop(inst.engine, excess[i:i + cap]))
                    si.on_wait = waits[-cap:]
                    inst.sync_info = si
                    changed = True
                out.append(inst)
            if changed:
                bb.instructions = out
    return nc


def _halves(n):
    """Split a psum free range into bank-aligned 512/256 fp32 pieces."""
    out, i = [], 0
    while i < n:
        step = 512 if n - i >= 512 else n - i
        out.append(slice(i, i + step))
        i += step
    return out


def build_kernel(N=768, D=768, H=16, HD=48, ZD=128, n_cores=N_CORES, QB=24, HG=4):
    _patch_tile_drain()
    NQL = N // n_cores          # 96 local queries
    KC = N // 128               # 6 k-chunks
    DC = D // 128               # 6 contraction chunks
    NQB = NQL // QB             # 4 z q-blocks per k-chunk
    NHG = H // HG               # 4 head groups
    assert NQL % QB == 0 and H % HG == 0

    nc = bass.Bass()

    s_full = nc.dram_tensor("s_full", [N, D], BF16, kind="ExternalInput")
    s_loc = nc.dram_tensor("s_loc", [NQL, D], BF16, kind="ExternalInput")
    # host-transposed z: [kc, qb, c, q*k]; flat last dim keeps DMA
    # descriptors >= 512B (smaller lines pay a 2x latency penalty)
    zq = nc.dram_tensor("zq", [KC, NQB, ZD, QB * 128], FP8,
                        kind="ExternalInput")
    zsq = nc.dram_tensor("zsq", [KC, NQB, ZD, QB * 128], FP8,
                         kind="ExternalInput")
    qw = nc.dram_tensor("qw", [D + 128, D], BF16, kind="ExternalInput")
    kw = nc.dram_tensor("kw", [D, D], BF16, kind="ExternalInput")
    vw = nc.dram_tensor("vw", [D, D], BF16, kind="ExternalInput")
    gw = nc.dram_tensor("gw", [D + 128, D], BF16, kind="ExternalInput")
    ow = nc.dram_tensor("ow", [D, D], BF16, kind="ExternalInput")
    w2 = nc.dram_tensor("w2", [ZD, H + 1], BF16, kind="ExternalInput")
    out = nc.dram_tensor("out", [NQL, D], F32, kind="ExternalOutput")

    with TileContext(nc) as tc, ExitStack() as top:
        consts = top.enter_context(tc.tile_pool(name="consts", bufs=1))
        persist = top.enter_context(tc.tile_pool(name="persist", bufs=1))

        # PSUM pools, LIFO-ordered for staged teardown
        uzp = top.enter_context(tc.tile_pool(name="uzp", bufs=2, space="PSUM"))

        ident = consts.tile([128, 128], BF16)
        make_identity(nc, ident)
        eps_sb = consts.tile([128, 1], F32)
        nc.vector.memset(eps_sb, EPS)
        scr = consts.tile([1, 2], F32)
        # preload the Exp ACT table (the only table this kernel ever needs)
        nc.scalar.activation(scr[:, 0:1], eps_sb[0:1], AF.Exp)
        shift_sb = consts.tile([128, 1], F32)
        nc.vector.memset(shift_sb, -EXP_SHIFT)
        ones_row = consts.tile([65, 96], BF16)
        nc.vector.memset(ones_row, 1.0)
        ones_col = consts.tile([128, 1], BF16)
        nc.vector.memset(ones_col, 1.0)
        w2_sb = consts.tile([ZD, H + 1], BF16)

        qT_sb = persist.tile([48, H, NQL], BF16)
        kT_sb = persist.tile([48, H, N], BF16)
        v_sb = persist.tile([128, KC, H, HD + 1], BF16)
        g_sb = persist.tile([NQL, D], BF16)
        nc.vector.memset(v_sb[:, :, :, HD], 1.0)  # ones col feeds sum(exp)

        # z-stream SBUF pools (chunk-granular tiles)
        ztp = top.enter_context(tc.tile_pool(name="ztp", bufs=4))
        zqp = top.enter_context(tc.tile_pool(name="zqp", bufs=2))
        statp = top.enter_context(tc.tile_pool(name="statp", bufs=2))
        ukcp = top.enter_context(tc.tile_pool(name="ukcp", bufs=2))
        kcp = top.enter_context(tc.tile_pool(name="kcp", bufs=2))

        # Phase-A SBUF pools (closed before the tail)
        pa_sb = ExitStack()
        wq_p = pa_sb.enter_context(tc.tile_pool(name="wq_p", bufs=1))
        wkv_p = pa_sb.enter_context(tc.tile_pool(name="wkv_p", bufs=1))
        apool = pa_sb.enter_context(tc.tile_pool(name="apool", bufs=1))
        asm = pa_sb.enter_context(tc.tile_pool(name="asm", bufs=1))
        raw_st = ExitStack()
        rawp = raw_st.enter_context(tc.tile_pool(name="rawp", bufs=1))
        corep = raw_st.enter_context(tc.tile_pool(name="corep", bufs=1))

        # ---------------- DMA kickoff ----------------
        # Two DMA mule queues (SP HWDGE, Pool SWDGE; Pool's exec queue lets
        # its own compute overlap in-flight transfers). ACT carries only
        # s_full, ahead of the first LN activation. Weight tensors ship with
        # their bias row appended as a 7th 128-row block (row 0 = bias).
        zc_tiles = {}
        zs_tiles = {}

        def z_dma(kc, eng, split=False):
            t = ztp.tile([ZD, NQB, QB * 128], FP8, tag="zt", name=f"zc{kc}")
            zc_tiles[kc] = t
            sv = zq.ap()[kc].rearrange("b c x -> c b x")
            if split:
                eng.dma_start(t[:, 0:2], sv[:, 0:2])
                eng.dma_start(t[:, 2:4], sv[:, 2:4])
            else:
                eng.dma_start(t, sv)

        def zsq_dma(kc, eng, split=False):
            t = zqp.tile([ZD, NQB, QB * 128], FP8, tag="zs", name=f"zs{kc}")
            zs_tiles[kc] = t
            sv = zsq.ap()[kc].rearrange("b c x -> c b x")
            if split:
                eng.dma_start(t[:, 0:2], sv[:, 0:2])
                eng.dma_start(t[:, 2:4], sv[:, 2:4])
            else:
                eng.dma_start(t, sv)

        # SP queue: local s, z0 early (it gates the first bias chain), qw,
        # then the front of the z stream; gw only feeds the post-loop gate
        raw_l = rawp.tile([NQL, D], BF16, tag="rawl")
        nc.sync.dma_start(raw_l, s_loc.ap())
        qw_sb = wq_p.tile([128, DC + 1, D], BF16)
        nc.sync.dma_start(qw_sb, qw.ap().rearrange("(c p) m -> p c m", p=128))
        qb_sb = qw_sb[0:1, DC]
        z_dma(0, nc.sync, split=True)
        # ACT queue: s_full only (ACT's exec queue depth is 0: a DMA
        # blocks every subsequent ACT activation for its transfer)
        raw_f = rawp.tile([128, KC, D], BF16, tag="rawf")
        sfv = s_full.ap().rearrange("(t p) m -> p t m", p=128)
        nc.scalar.dma_start(raw_f[:, 0:3], sfv[:, 0:3])
        nc.scalar.dma_start(raw_f[:, 3:6], sfv[:, 3:6])
        # Pool queue
        nc.gpsimd.dma_start(w2_sb, w2.ap())
        kw_sb = wkv_p.tile([128, DC, D], BF16)
        nc.gpsimd.dma_start(kw_sb, kw.ap().rearrange("(c p) m -> p c m", p=128))
        zsq_dma(0, nc.gpsimd, split=True)
        zsq_dma(1, nc.gpsimd)
        vw_sb = wkv_p.tile([128, DC, D], BF16)
        nc.gpsimd.dma_start(vw_sb, vw.ap().rearrange("(c p) m -> p c m", p=128))
        z_dma(1, nc.sync)
        z_dma(2, nc.sync)
        zsq_dma(2, nc.gpsimd)
        z_dma(3, nc.sync)
        zsq_dma(3, nc.gpsimd)
        zsq_dma(4, nc.sync)
        z_dma(4, nc.gpsimd)
        zsq_dma(5, nc.sync)
        z_dma(5, nc.gpsimd)
        gw_sb = wq_p.tile([128, DC + 1, D], BF16)
        nc.sync.dma_start(gw_sb, gw.ap().rearrange("(c p) m -> p c m", p=128))
        gb_sb = gw_sb[0:1, DC]

        # ---------------- LN(s) ----------------
        def ln_core(raw_list, rows, name):
            nt = len(raw_list)
            core = corep.tile([128, nt, D], BF16, tag=f"core_{name}",
                              name=f"core_{name}")
            SG = 2
            st = asm.tile([128, nt, SG, 6], F32, tag=f"st_{name}")
            mv = asm.tile([128, nt, 2], F32, tag=f"mv_{name}")
            neg = asm.tile([128, nt], F32, tag=f"ng_{name}")
            rr = asm.tile([128, nt], F32, tag=f"rr_{name}")
            vv = asm.tile([128, nt], F32, tag=f"vv_{name}")
            y0 = asm.tile([128, nt], F32, tag=f"y0_{name}")
            for t in range(nt):
                p = rows - t * 128 if (t == nt - 1 and rows % 128) else 128
                rt = raw_list[t][:p]
                rv = rt.rearrange("p (g x) -> p g x", g=SG)
                for g in range(SG):
                    nc.vector.bn_stats(st[:p, t, g], rv[:, g])
                nc.vector.bn_aggr(mv[:p, t], st[:p, t])
                # table-free rstd: one Newton step from y0 = 0.5 + 0.5/v
                # (exact at v=1; LN variance of ~unit-normal rows is ~1)
                tsl_ = slice(t, t + 1)
                nc.vector.tensor_scalar(vv[:p, tsl_], mv[:p, t, 1:2], EPS,
                                        None, ALU.add)
                nc.vector.reciprocal(y0[:p, tsl_], vv[:p, tsl_])
                nc.vector.tensor_scalar(y0[:p, tsl_], y0[:p, tsl_], 0.5, 0.5,
                                        ALU.mult, ALU.add)
                nc.vector.tensor_tensor(rr[:p, tsl_], vv[:p, tsl_],
                                        y0[:p, tsl_], ALU.mult)
                nc.vector.scalar_tensor_tensor(out=rr[:p, tsl_],
                                               in0=rr[:p, tsl_], scalar=-0.5,
                                               in1=y0[:p, tsl_],
                                               op0=ALU.mult, op1=ALU.mult)
                nc.vector.scalar_tensor_tensor(out=rr[:p, tsl_],
                                               in0=rr[:p, tsl_], scalar=1.5,
                                               in1=y0[:p, tsl_],
                                               op0=ALU.add, op1=ALU.mult)
                nc.vector.tensor_tensor(neg[:p, tsl_], mv[:p, t, 0:1],
                                        rr[:p, tsl_], ALU.mult)
                nc.vector.tensor_scalar(neg[:p, tsl_], neg[:p, tsl_],
                                        -1.0, None, ALU.mult)
                if t % 2 == 0:
                    nc.scalar.activation(core[:p, t], rt, AF.Identity,
                                         bias=neg[:p, tsl_],
                                         scale=rr[:p, tsl_])
                else:
                    nc.vector.tensor_scalar(core[:p, t], rt, rr[:p, tsl_],
                                            neg[:p, tsl_], ALU.mult, ALU.add)
            return core

        core_l = ln_core([raw_l], NQL, "l")
        core = ln_core([raw_f[:, t] for t in range(KC)], N, "f")

        # ---------------- z helpers ----------------

        def z_tile_pe(kc, qb):
            """u8 + ssq matmuls for z tile (kc, qb); shared psum bank."""
            zt = zc_tiles[kc][:, qb].rearrange("c (q k) -> c q k", q=QB)
            zs = zs_tiles[kc][:, qb].rearrange("c (q k) -> c q k", q=QB)
            bank = uzp.tile([128, 512], F32, tag="uz")
            u8 = bank[:, :QB * (H + 1)].rearrange("p (q h) -> p q h", h=H + 1)
            ssq = bank[:, QB * (H + 1):QB * (H + 2)]
            for j in range(QB):
                nc.tensor.matmul(u8[:, j], zt[:, j], w2_sb,
                                 start=(j == 0), stop=False)
            for j in range(QB):
                nc.tensor.matmul(ssq[:, j:j + 1], zs[:, j], ones_col,
                                 start=False, stop=(j == QB - 1))
            return u8, ssq

        def z_pair_stats(pair, u_kc):
            """LN stats for a pair of tiles (both psum banks live). rstd via
            a table-free DVE Newton step: y0 = 0.5 + 0.5/v (exact at v=1),
            rstd ~= y0*(1.5 - 0.5*v*y0^2). u_kc is [k, h, q] so the PE's
            bias-accumulate matmuls read contiguous per-head columns."""
            P2 = QB * len(pair)
            q0 = (pair[0][0] % NQB) * QB
            varb = statp.tile([128, P2], F32, tag="varb")
            for i, (ti, u8, ssq) in enumerate(pair):
                mu2 = statp.tile([128, QB], F32, tag="mu2")
                nc.scalar.square(mu2, u8[:, :, H])
                nc.vector.scalar_tensor_tensor(
                    out=varb[:, i * QB:(i + 1) * QB], in0=ssq,
                    scalar=1.0 / ZD, in1=mu2, op0=ALU.mult, op1=ALU.subtract)
            y0 = statp.tile([128, P2], F32, tag="y0")
            nc.vector.reciprocal(y0, varb)
            nc.vector.tensor_scalar(y0, y0, 0.5, 0.5, ALU.mult, ALU.add)
            t1 = statp.tile([128, P2], F32, tag="t1")
            nc.vector.tensor_tensor(t1, varb, y0, ALU.mult)
            nc.vector.scalar_tensor_tensor(out=t1, in0=t1, scalar=-0.5,
                                           in1=y0, op0=ALU.mult, op1=ALU.mult)
            nc.vector.scalar_tensor_tensor(out=t1, in0=t1, scalar=1.5,
                                           in1=y0, op0=ALU.add, op1=ALU.mult)
            for i, (ti, u8, ssq) in enumerate(pair):
                qs = slice(q0 + i * QB, q0 + (i + 1) * QB)
                nc.vector.tensor_tensor(
                    u_kc[:, :, qs].rearrange("k h q -> k q h"), u8[:, :, :H],
                    t1[:, i * QB:(i + 1) * QB, None].to_broadcast(
                        [128, QB, H]), ALU.mult)

        def emit_zpair(kc, half, u_kc):
            pair = []
            for qb in (2 * half, 2 * half + 1):
                u8, ssq = z_tile_pe(kc, qb)
                pair.append((kc * NQB + qb, u8, ssq))
            z_pair_stats(pair, u_kc)

        def emit_zproj(kc):
            u_kc = ukcp.tile([128, H, NQL], BF16, tag="ukc")
            for half in range(2):
                emit_zpair(kc, half, u_kc)
            return u_kc


        def kproj_kt(t, act_copy=False):
            tsl = slice(t * 128, (t + 1) * 128)
            k_sb = apool.tile([128, D], BF16, tag="knat")
            for sl in _halves(D):
                nn_ = sl.stop - sl.start
                ps = aps.tile([128, 512], F32, tag="proj")
                for c in range(DC):
                    nc.tensor.matmul(ps[:, :nn_], snT[:, c, tsl],
                                     kw_sb[:, c, sl], start=(c == 0),
                                     stop=(c == DC - 1))
                nc.scalar.copy(k_sb[:, sl], ps[:, :nn_])  # ACT: psum evac
            for hc in range(2):
                kt_ps = uzp.tile([48, 8, 128], BF16, tag="uz", name="kt_ps")
                for hh in range(8):
                    h = hc * 8 + hh
                    nc.tensor.transpose(kt_ps[:, hh],
                                        k_sb[:, h * HD:(h + 1) * HD], ident)
                if act_copy:
                    nc.scalar.copy(kT_sb[:, hc * 8:(hc + 1) * 8, tsl], kt_ps)
                else:
                    nc.vector.tensor_copy(kT_sb[:, hc * 8:(hc + 1) * 8, tsl],
                                          kt_ps)

        def vproj(t):
            tsl = slice(t * 128, (t + 1) * 128)
            for sl in (slice(0, 480), slice(480, 768)):
                nn_ = sl.stop - sl.start
                h0 = sl.start // HD
                nh = nn_ // HD
                ps = aps.tile([128, 512], F32, tag="proj")
                for c in range(DC):
                    nc.tensor.matmul(ps[:, :nn_], snT[:, c, tsl],
                                     vw_sb[:, c, sl], start=(c == 0),
                                     stop=(c == DC - 1))
                nc.scalar.copy(v_sb[:, t, h0:h0 + nh, :HD],
                               ps[:, :nn_].rearrange("p (h d) -> p h d", d=HD))

        # psum pools for the lead-in + loop: uzp(2) + sps(2) + aps(2) +
        # apt1(2) = 8 banks; apt1 closes before ops_ (o accumulator) opens,
        # and the tail reuses the aps ring, keeping the pool trace stack-LIFO.
        sps = top.enter_context(tc.tile_pool(name="sps", bufs=2, space="PSUM"))
        aps = top.enter_context(tc.tile_pool(name="aps", bufs=2, space="PSUM"))

        # snT: transpose LN'd s to [d, tok], per-(d,t) 128x128 blocks so the
        # k pipeline can consume token-chunk 0 as soon as it exists
        snT = apool.tile([128, DC, N], BF16)
        snT_loc = apool.tile([128, DC, NQL], BF16)
        with tc.tile_pool(name="apt1", bufs=1, space="PSUM") as apt1:
            ps_l = apt1.tile([128, DC, NQL], BF16, tag="tpl",
                             padded_shape=[128, KC, 128])
            for d in range(DC):
                nc.tensor.transpose(ps_l[:, d],
                                    core_l[:NQL, 0, d * 128:(d + 1) * 128],
                                    ident[:NQL, :NQL])
            nc.vector.tensor_copy(snT_loc, ps_l)
            tpd = apt1.tile([128, DC, 128], BF16, tag="tpd")
            ci = 0
            for t in range(KC):
                tsl = slice(t * 128, (t + 1) * 128)
                for d in range(DC):
                    nc.tensor.transpose(tpd[:, d],
                                        core[:, t, d * 128:(d + 1) * 128],
                                        ident)
                    if ci % 2 == 0:
                        nc.scalar.copy(snT[:, d, tsl], tpd[:, d])
                    else:
                        nc.vector.tensor_copy(snT[:, d, tsl], tpd[:, d])
                    ci += 1
                if t == 0:
                    # q local projection + qT fill the wait for zc0's DMA
                    q_sb = apool.tile([NQL, D], BF16, tag="qnat")
                    for sl in _halves(D):
                        nn_ = sl.stop - sl.start
                        ps = aps.tile([128, 512], F32, tag="proj")
                        nc.tensor.matmul(ps[:NQL, :nn_], ones_row[0:1, :NQL],
                                         qb_sb[:, sl], start=True, stop=False)
                        for c in range(DC):
                            nc.tensor.matmul(ps[:NQL, :nn_], snT_loc[:, c],
                                             qw_sb[:, c, sl], start=False,
                                             stop=(c == DC - 1))
                        nc.vector.tensor_copy(q_sb[:, sl], ps[:NQL, :nn_])
                    for hc in range(2):
                        qt_ps = uzp.tile([48, 8, 128], BF16, tag="uz",
                                         name="qt_ps")
                        for hh in range(8):
                            h = hc * 8 + hh
                            nc.tensor.transpose(qt_ps[:, hh, :NQL],
                                                q_sb[:, h * HD:(h + 1) * HD],
                                                ident[:NQL, :NQL])
                        nc.vector.tensor_copy(qT_sb[:, hc * 8:(hc + 1) * 8],
                                              qt_ps[:, :, :NQL])
                    u_kcs = {0: emit_zproj(0)}
                    kproj_kt(0)
                if t == 1:
                    kproj_kt(1)

        raw_st.close()
        ops_ = top.enter_context(tc.tile_pool(name="ops", bufs=1, space="PSUM"))
        o_ps = ops_.tile([NQL, H, 64], F32)
        # o_ps starts from vb (broadcast over queries): softmax rows sum to 1,
        # so adding vb once up front equals adding it to every v row.
        vbp_a = qw_sb[32:33, DC, 0:512]
        vbp_b = qw_sb[64:65, DC, 0:512]
        nc.tensor.matmul(o_ps[:, 0:8], ones_row[32:33, :NQL], vbp_a,
                         start=True, stop=False)
        nc.tensor.matmul(o_ps[:, 8:16], ones_row[64:65, :NQL], vbp_b,
                         start=True, stop=False)

        def scores_exp_kc(kc, u_kc):
            """scores + pair-bias accumulate on PE, exp from PSUM on ACT."""
            ksl = slice(kc * 128, (kc + 1) * 128)
            p_sb = kcp.tile([128, H, NQL], BF16, tag="psb")
            for hg in range(NHG):
                h0 = hg * HG
                s_ps = sps.tile([128, 512], F32, tag="sps")
                s4 = s_ps[:, :HG * NQL].rearrange("p (h q) -> p h q", q=NQL)
                for hh in range(HG):
                    h = h0 + hh
                    nc.tensor.matmul(s4[:, hh], kT_sb[:, h, ksl], qT_sb[:, h],
                                     start=(hh == 0), stop=False)
                for hh in range(HG):
                    h = h0 + hh
                    nc.tensor.matmul(s4[:, hh], ident, u_kc[:, h],
                                     start=False, stop=(hh == HG - 1))
                nc.scalar.activation(p_sb[:, h0:h0 + HG], s4, AF.Exp,
                                     bias=shift_sb)
            return p_sb

        def pv_kc(kc, p_sb):
            for h in range(H):
                nc.tensor.matmul(o_ps[:, h, :HD + 1], p_sb[:, h],
                                 v_sb[:, kc, h, :],
                                 start=False,
                                 stop=(kc == KC - 1 and h % 8 == 7))

        # -------- main pipelined loop over k-chunks --------
        # k projection runs two chunks ahead so kT(t) is already in SBUF when
        # scores(t) issue; the per-chunk critical chain is then just
        # u_kc(t) -> scores(t) -> exp(t) -> PV(t).
        pend = {}
        for t in range(KC):
            # first half of the next chunk's z projection goes ahead of the
            # scores so its DVE stats overlap this chunk's exp/PV
            if t + 1 < KC:
                u_next = ukcp.tile([128, H, NQL], BF16, tag="ukc",
                                   name="u_next")
                emit_zpair(t + 1, 0, u_next)
            pend[t] = scores_exp_kc(t, u_kcs.pop(t))
            if t >= 1:
                pv_kc(t - 1, pend.pop(t - 1))
            if t + 1 < KC:
                emit_zpair(t + 1, 1, u_next)
                u_kcs[t + 1] = u_next
            if t + 2 < KC:
                kproj_kt(t + 2)
            if t == 4:
                ow_sb = persist.tile([128, DC, D], BF16)
                nc.gpsimd.dma_start(ow_sb,
                                    ow.ap().rearrange("(c p) m -> p c m",
                                                      p=128))
            vproj(t)
        pv_kc(KC - 1, pend.pop(KC - 1))

        # g projection (off the critical path; gw arrived long ago)
        for sl in _halves(D):
            nn_ = sl.stop - sl.start
            ps = aps.tile([128, 512], F32, tag="proj")
            nc.tensor.matmul(ps[:NQL, :nn_], ones_row[:, :NQL],
                             gb_sb[:, sl], start=True, stop=False)
            for c in range(DC):
                nc.tensor.matmul(ps[:NQL, :nn_], snT_loc[:, c],
                                 gw_sb[:, c, sl], start=False,
                                 stop=(c == DC - 1))
            nc.scalar.activation(g_sb[:, sl], ps[:NQL, :nn_], AF.Exp,
                                 scale=-1.0)
        # g = sigmoid(x) = 1 / (1 + exp(-x)), finished on DVE
        with nc.allow_low_precision(reason="bf16 gate: 1+exp(-x) then recip"):
            nc.vector.tensor_scalar(g_sb, g_sb, 1.0, None, ALU.add)
            nc.vector.reciprocal(g_sb, g_sb)

        # ================= tail: normalize, gate, project =================
        pa_sb.close()
        tail = top.enter_context(tc.tile_pool(name="tail", bufs=1))
        rcp = tail.tile([NQL, H], F32)
        o_sb = tail.tile([NQL, H, HD], BF16)
        og = tail.tile([NQL, D], BF16)
        ogt = tail.tile([128, DC, NQL], BF16)
        out_ps = [aps.tile([NQL, 512], F32, tag="proj", name=f"out_ps{i}")
                  for i in range(2)]
        out_sb = [tail.tile([NQL, 512], F32, name=f"out_sb{i}")
                  for i in range(2)]
        # head-half pipelining: o_ps bank 0 (heads 0-7) finishes PV first, so
        # its normalize/gate/transpose overlaps the second bank's PV drain,
        # and the out projection accumulates c-chunks as they appear
        for hb in range(2):
            hs = slice(hb * 8, hb * 8 + 8)
            nc.vector.reciprocal(rcp[:, hs], o_ps[:, hs, HD])
            nc.vector.tensor_tensor(o_sb[:, hs], o_ps[:, hs, :HD],
                                    rcp[:, hs, None].to_broadcast(
                                        [NQL, 8, HD]), ALU.mult)
            csl = slice(hb * 384, hb * 384 + 384)
            nc.gpsimd.tensor_mul(og[:, csl],
                                  o_sb.rearrange("q h d -> q (h d)")[:, csl],
                                  g_sb[:, csl])
            ogt_ps = uzp.tile([128, 3, NQL], BF16, tag="uz",
                              name=f"ogt_ps{hb}")
            for dd in range(3):
                d = hb * 3 + dd
                nc.tensor.transpose(ogt_ps[:, dd],
                                    og[:, d * 128:(d + 1) * 128],
                                    ident[:NQL, :NQL])
            nc.vector.tensor_copy(ogt[:, hb * 3:hb * 3 + 3], ogt_ps)
            for i, sl in enumerate(_halves(D)):
                nn_ = sl.stop - sl.start
                for dd in range(3):
                    c = hb * 3 + dd
                    nc.tensor.matmul(out_ps[i][:, :nn_], ogt[:, c],
                                     ow_sb[:, c, sl], start=(c == 0),
                                     stop=(c == DC - 1))
        for i, sl in enumerate(_halves(D)):
            nn_ = sl.stop - sl.start
            if i == 0:
                nc.scalar.copy(out_sb[i][:, :nn_], out_ps[i][:, :nn_])
            else:
                nc.vector.tensor_copy(out_sb[i][:, :nn_], out_ps[i][:, :nn_])
            eng = nc.sync if i == 0 else nc.scalar
            eng.dma_start(out.ap()[:, sl], out_sb[i][:, :nn_])

    _split_excess_waits(nc)
    return nc


def prep_inputs(inputs, N=768, D=768, H=16, HD=48, ZD=128, n_cores=N_CORES,
                QB=24):
    """Host-side: fold LN(s) weights + scale into projections; cast z to
    bf16 / z^2 to fp8 and transpose per core to [kc, qb, c, q, k]."""
    f32 = np.float32
    bf16 = ml_dtypes.bfloat16
    fp8 = ml_dtypes.float8_e4m3
    s = np.asarray(inputs["s"], f32).reshape(N, D)
    z = np.asarray(inputs["z"], f32).reshape(N, N, ZD)
    wv = np.asarray(inputs["norm_s_w"], f32)
    bv = np.asarray(inputs["norm_s_b"], f32)
    scale = HD ** -0.5

    def fold(wm, bias_extra=None, sc=1.0):
        wm = np.asarray(wm, f32)
        wf = (wv[:, None] * wm) * sc
        bf = (bv @ wm) * sc
        if bias_extra is not None:
            bf = bf + np.asarray(bias_extra, f32) * sc
        return wf, bf

    qwf, qbf = fold(inputs["q_w"], inputs["q_b"], scale)
    kwf, _ = fold(inputs["k_w"])          # k bias is softmax-invariant: drop
    vwf, vbf = fold(inputs["v_w"])
    gwf, gbf = fold(inputs["g_w"])
    W2 = (np.asarray(inputs["z_norm_w"], f32)[:, None]
          * np.asarray(inputs["z_w"], f32))
    alpha = W2.sum(axis=0)
    W2aug = np.concatenate(
        [W2 - alpha[None, :] / ZD, np.full((ZD, 1), 1.0 / ZD, f32)], axis=1)

    zb = z.astype(fp8)
    zsqb = (z * z).astype(fp8)
    sb = s.astype(bf16)
    NQL = N // n_cores
    KC = N // 128
    NQB = NQL // QB

    def with_bias(wf, bf, extra=None):
        ext = np.zeros((128, D), f32)
        ext[0] = bf
        if extra is not None:
            ext[32] = extra[:D]
            ext[64, :len(extra) - D] = extra[D:]
        return np.concatenate([wf, ext]).astype(bf16)

    vb_pad = np.zeros((16, 64), f32)
    vb_pad[:, :HD] = vbf.reshape(H, HD)
    shared = {
        "s_full": sb,
        "qw": with_bias(qwf, qbf, vb_pad.ravel()), "kw": kwf.astype(bf16),
        "vw": vwf.astype(bf16), "gw": with_bias(gwf, gbf),
        "ow": np.asarray(inputs["o_w"], f32).astype(bf16),
        "w2": W2aug.astype(bf16),
    }

    def zshape(a):
        zt = a.reshape(NQB, QB, KC, 128, ZD)                 # qb qq kc kk c
        zt = np.ascontiguousarray(zt.transpose(2, 0, 4, 1, 3))
        return zt.reshape(KC, NQB, ZD, QB * 128)

    in_maps = []
    for d in range(n_cores):
        m = dict(shared)
        m["s_loc"] = np.ascontiguousarray(sb[d * NQL:(d + 1) * NQL])
        m["zq"] = zshape(zb[d * NQL:(d + 1) * NQL])
        m["zsq"] = zshape(zsqb[d * NQL:(d + 1) * NQL])
        in_maps.append(m)
    return in_maps


_CACHED = {}


def kernel(**inputs) -> np.ndarray:
    from concourse.bass_utils import run_bass_kernel_spmd
    N, D = 768, 768
    if "nc" not in _CACHED:
        _CACHED["nc"] = build_kernel()
    nc = _CACHED["nc"]
    in_maps = prep_inputs(inputs)
    res = run_bass_kernel_spmd(nc, in_maps, core_ids=list(range(N_CORES)))
    _CACHED["last_result"] = res
    out = np.concatenate([r["out"] for r in res.results], axis=0)
    return out.reshape(1, N, D)


# revision 56
# speedup vs baseline: 1.0012x; 1.0012x over previous
"""AttentionPairBias Trainium2 Bass kernel, 8-way query-sharded.

Per core (N=768, D=768, H=16, HD=48, ZD=128): core d owns query rows
[d*96,(d+1)*96). z AND z^2 arrive host-cast to fp8e4m3 (the fp8 z^2
stream replaces all on-device squaring; LN statistics still reduce on
device), host-transposed to [kc, qb, c=128, q=24, k=128] so every DMA
line is contiguous and the ZD contraction dim is on partitions. Per
z tile the PE does the whole LN reduction with per-q-column
stationaries: u8[k,q,17] = zt_q^T @ W2' (mean rides as a ones/128
column, mixed fp8xbf16 matmul) and ssq[k,q] = zsq_q^T @ ones. Both
rstd paths (z-bias and LN(s)) use a table-free DVE Newton step, so Exp
is the only ACT table ever loaded (preloaded at kernel start).
LayerNorm folds: W2' = z_norm_w*z_w - ones*colsum/128; the k-side
projection bias is dropped (softmax shift invariance); LN(s)
weight/bias, q_b and the 1/sqrt(HD) scale fold into projection weights
host-side; s ships as bf16; each projection bias rides as row 0 of an
extra 128-row block of its weight tensor; v_b is folded into the o
accumulator via two rank-1 PSUM matmuls (softmax rows sum to 1).

The pair bias is accumulated into the scores PSUM by identity matmuls
on the PE (u_kc is materialized [k,h,q] so the moving operand is
contiguous per head), and exp reads the PSUM bank directly on ACT.
DMA issue is balanced over the SP HWDGE and Pool SWDGE queues (a DMA
occupies its issuing engine for the whole transfer in this machine
model; ACT carries only s_full since its exec-queue depth of 0 would
stall activations). The main loop is software-pipelined: k projection
runs two chunks ahead, the first half of chunk t+1's z projection is
emitted before chunk t's scores so its DVE stats overlap exp/PV, and
the lead-in interleaves per-(d,t) snT transpose blocks with the k/z
pipelines so the PE never queues behind a not-yet-normalized token
chunk. The tail pipelines normalize/gate/transpose/project by head
halves and splits the output DMA across the SP and ACT queues.
"""

from contextlib import ExitStack

import numpy as np
import ml_dtypes

import concourse.bass as bass
import concourse.mybir as mybir
from concourse.tile import TileContext
from concourse.vector_clock import ScopedClock
from concourse.masks import make_identity

F32 = mybir.dt.float32
BF16 = mybir.dt.bfloat16
FP8 = mybir.dt.float8e4
AF = mybir.ActivationFunctionType
ALU = mybir.AluOpType

N_CORES = 8
EPS = 1e-5
EXP_SHIFT = 3.0


def _patch_tile_drain():
    """walrus in this container caps sync waits per CTRL instruction; spread
    the TileContext tail-drain waits across single-wait SP nops."""
    if getattr(TileContext, "_drain_patched", False):
        return

    def _drain_and_barrier(self, tick_clock, wait_clock):
        nc = self.nc
        probe = nc.sync.nop(nofuse=True, hint="tail_wait_probe")
        wait_clock.add_sem_waits(probe.ins, ScopedClock({None: tick_clock.global_clock}))
        si = probe.ins.sync_info
        waits = list(si.on_wait or []) if si else []
        if len(waits) > 1:
            si.on_wait = waits[:1]
            for w in waits[1:]:
                n2 = nc.sync.nop(nofuse=True, hint="tail_wait_split")
                n2.ins.sync_info = mybir.SyncInfo(on_wait=[w], on_update=[])
        nc.sync.drain()
        nc.all_engine_barrier()
        assert self.sems is not None
        popped = nc._tile_sem_poison_stack.pop()
        assert popped is self._sem_poison
        nc.clear_and_free_semaphores(list(self.sems.allocated().values()))
        nc.all_engine_barrier()

    TileContext._drain_and_barrier = _drain_and_barrier
    TileContext._drain_patched = True


def _split_excess_waits(nc, cap=1):
    """walrus in this container rejects instructions with more than ~2 sync
    waits; move the excess onto same-engine NOPs placed just before."""
    ctr = [0]

    def mk_nop(engine, waits):
        ctr[0] += 1
        nop = mybir.InstNoOp(name=f"I-waitsplit-{ctr[0]}", ins=[], outs=[])
        nop.engine = engine
        nop.sync_info = mybir.SyncInfo(on_wait=waits, on_update=[])
        return nop

    for f in nc.m.functions:
        for bb in f.blocks:
            out, changed = [], False
            for inst in bb.instructions:
                si = inst.sync_info
                waits = list(si.on_wait) if si and si.on_wait else []
                if len(waits) > cap:
                    excess = waits[:-cap]
                    for i in range(0, len(excess), cap):
                        out.append(mk_nop(inst.engine, excess[i:i + cap]))
                    si.on_wait = waits[-cap:]
                    inst.sync_info = si
                    changed = True
                out.append(inst)
            if changed:
                bb.instructions = out
    return nc


def _halves(n):
    """Split a psum free range into bank-aligned 512/256 fp32 pieces."""
    out, i = [], 0
    while i < n:
        step = 512 if n - i >= 512 else n - i
        out.append(slice(i, i + step))
        i += step
    return out


def build_kernel(N=768, D=768, H=16, HD=48, ZD=128, n_cores=N_CORES, QB=24, HG=4):
    _patch_tile_drain()
    NQL = N // n_cores          # 96 local queries
    KC = N // 128               # 6 k-chunks
    DC = D // 128               # 6 contraction chunks
    NQB = NQL // QB             # 4 z q-blocks per k-chunk
    NHG = H // HG               # 4 head groups
    assert NQL % QB == 0 and H % HG == 0

    nc = bass.Bass()

    s_full = nc.dram_tensor("s_full", [N, D], BF16, kind="ExternalInput")
    s_loc = nc.dram_tensor("s_loc", [NQL, D], BF16, kind="ExternalInput")
    # host-transposed z: [kc, qb, c, q*k]; flat last dim keeps DMA
    # descriptors >= 512B (smaller lines pay a 2x latency penalty)
    zq = nc.dram_tensor("zq", [KC, NQB, ZD, QB * 128], FP8,
                        kind="ExternalInput")
    zsq = nc.dram_tensor("zsq", [KC, NQB, ZD, QB * 128], FP8,
                         kind="ExternalInput")
    qw = nc.dram_tensor("qw", [D + 128, D], BF16, kind="ExternalInput")
    kw = nc.dram_tensor("kw", [D, D], BF16, kind="ExternalInput")
    vw = nc.dram_tensor("vw", [D, D], BF16, kind="ExternalInput")
    gw = nc.dram_tensor("gw", [D + 128, D], BF16, kind="ExternalInput")
    ow = nc.dram_tensor("ow", [D, D], BF16, kind="ExternalInput")
    w2 = nc.dram_tensor("w2", [ZD, H + 1], BF16, kind="ExternalInput")
    out = nc.dram_tensor("out", [NQL, D], BF16, kind="ExternalOutput")

    with TileContext(nc) as tc, ExitStack() as top:
        consts = top.enter_context(tc.tile_pool(name="consts", bufs=1))
        persist = top.enter_context(tc.tile_pool(name="persist", bufs=1))

        # PSUM pools, LIFO-ordered for staged teardown
        uzp = top.enter_context(tc.tile_pool(name="uzp", bufs=2, space="PSUM"))

        ident = consts.tile([128, 128], BF16)
        make_identity(nc, ident)
        eps_sb = consts.tile([128, 1], F32)
        nc.vector.memset(eps_sb, EPS)
        scr = consts.tile([1, 2], F32)
        # preload the Exp ACT table (the only table this kernel ever needs)
        nc.scalar.activation(scr[:, 0:1], eps_sb[0:1], AF.Exp)
        shift_sb = consts.tile([128, 1], F32)
        nc.vector.memset(shift_sb, -EXP_SHIFT)
        ones_row = consts.tile([65, 96], BF16)
        nc.vector.memset(ones_row, 1.0)
        ones_col = consts.tile([128, 1], BF16)
        nc.vector.memset(ones_col, 1.0)
        w2_sb = consts.tile([ZD, H + 1], BF16)

        qT_sb = persist.tile([48, H, NQL], BF16)
        kT_sb = persist.tile([48, H, N], BF16)
        v_sb = persist.tile([128, KC, H, HD + 1], BF16)
        g_sb = persist.tile([NQL, D], BF16)
        nc.vector.memset(v_sb[:, :, :, HD], 1.0)  # ones col feeds sum(exp)

        # z-stream SBUF pools (chunk-granular tiles)
        ztp = top.enter_context(tc.tile_pool(name="ztp", bufs=4))
        zqp = top.enter_context(tc.tile_pool(name="zqp", bufs=2))
        statp = top.enter_context(tc.tile_pool(name="statp", bufs=2))
        ukcp = top.enter_context(tc.tile_pool(name="ukcp", bufs=2))
        kcp = top.enter_context(tc.tile_pool(name="kcp", bufs=2))

        # Phase-A SBUF pools (closed before the tail)
        pa_sb = ExitStack()
        wq_p = pa_sb.enter_context(tc.tile_pool(name="wq_p", bufs=1))
        wkv_p = pa_sb.enter_context(tc.tile_pool(name="wkv_p", bufs=1))
        apool = pa_sb.enter_context(tc.tile_pool(name="apool", bufs=1))
        asm = pa_sb.enter_context(tc.tile_pool(name="asm", bufs=1))
        raw_st = ExitStack()
        rawp = raw_st.enter_context(tc.tile_pool(name="rawp", bufs=1))
        corep = raw_st.enter_context(tc.tile_pool(name="corep", bufs=1))

        # ---------------- DMA kickoff ----------------
        # Two DMA mule queues (SP HWDGE, Pool SWDGE; Pool's exec queue lets
        # its own compute overlap in-flight transfers). ACT carries only
        # s_full, ahead of the first LN activation. Weight tensors ship with
        # their bias row appended as a 7th 128-row block (row 0 = bias).
        zc_tiles = {}
        zs_tiles = {}

        def z_dma(kc, eng, split=False):
            t = ztp.tile([ZD, NQB, QB * 128], FP8, tag="zt", name=f"zc{kc}")
            zc_tiles[kc] = t
            sv = zq.ap()[kc].rearrange("b c x -> c b x")
            if split:
                eng.dma_start(t[:, 0:2], sv[:, 0:2])
                eng.dma_start(t[:, 2:4], sv[:, 2:4])
            else:
                eng.dma_start(t, sv)

        def zsq_dma(kc, eng, split=False):
            t = zqp.tile([ZD, NQB, QB * 128], FP8, tag="zs", name=f"zs{kc}")
            zs_tiles[kc] = t
            sv = zsq.ap()[kc].rearrange("b c x -> c b x")
            if split:
                eng.dma_start(t[:, 0:2], sv[:, 0:2])
                eng.dma_start(t[:, 2:4], sv[:, 2:4])
            else:
                eng.dma_start(t, sv)

        # SP queue: local s, z0 early (it gates the first bias chain), qw,
        # then the front of the z stream; gw only feeds the post-loop gate
        raw_l = rawp.tile([NQL, D], BF16, tag="rawl")
        nc.sync.dma_start(raw_l, s_loc.ap())
        qw_sb = wq_p.tile([128, DC + 1, D], BF16)
        nc.sync.dma_start(qw_sb, qw.ap().rearrange("(c p) m -> p c m", p=128))
        qb_sb = qw_sb[0:1, DC]
        z_dma(0, nc.sync, split=True)
        # ACT queue: s_full only (ACT's exec queue depth is 0: a DMA
        # blocks every subsequent ACT activation for its transfer)
        raw_f = rawp.tile([128, KC, D], BF16, tag="rawf")
        sfv = s_full.ap().rearrange("(t p) m -> p t m", p=128)
        nc.scalar.dma_start(raw_f[:, 0:3], sfv[:, 0:3])
        nc.scalar.dma_start(raw_f[:, 3:6], sfv[:, 3:6])
        # Pool queue
        nc.gpsimd.dma_start(w2_sb, w2.ap())
        kw_sb = wkv_p.tile([128, DC, D], BF16)
        nc.gpsimd.dma_start(kw_sb, kw.ap().rearrange("(c p) m -> p c m", p=128))
        zsq_dma(0, nc.gpsimd, split=True)
        zsq_dma(1, nc.gpsimd)
        vw_sb = wkv_p.tile([128, DC, D], BF16)
        nc.gpsimd.dma_start(vw_sb, vw.ap().rearrange("(c p) m -> p c m", p=128))
        z_dma(1, nc.sync)
        z_dma(2, nc.sync)
        zsq_dma(2, nc.gpsimd)
        z_dma(3, nc.sync)
        zsq_dma(3, nc.gpsimd)
        zsq_dma(4, nc.sync)
        z_dma(4, nc.gpsimd)
        zsq_dma(5, nc.sync)
        z_dma(5, nc.gpsimd)
        gw_sb = wq_p.tile([128, DC + 1, D], BF16)
        nc.sync.dma_start(gw_sb, gw.ap().rearrange("(c p) m -> p c m", p=128))
        gb_sb = gw_sb[0:1, DC]

        # ---------------- LN(s) ----------------
        def ln_core(raw_list, rows, name):
            nt = len(raw_list)
            core = corep.tile([128, nt, D], BF16, tag=f"core_{name}",
                              name=f"core_{name}")
            SG = 2
            st = asm.tile([128, nt, SG, 6], F32, tag=f"st_{name}")
            mv = asm.tile([128, nt, 2], F32, tag=f"mv_{name}")
            neg = asm.tile([128, nt], F32, tag=f"ng_{name}")
            rr = asm.tile([128, nt], F32, tag=f"rr_{name}")
            vv = asm.tile([128, nt], F32, tag=f"vv_{name}")
            y0 = asm.tile([128, nt], F32, tag=f"y0_{name}")
            for t in range(nt):
                p = rows - t * 128 if (t == nt - 1 and rows % 128) else 128
                rt = raw_list[t][:p]
                rv = rt.rearrange("p (g x) -> p g x", g=SG)
                for g in range(SG):
                    nc.vector.bn_stats(st[:p, t, g], rv[:, g])
                nc.vector.bn_aggr(mv[:p, t], st[:p, t])
                # table-free rstd: one Newton step from y0 = 0.5 + 0.5/v
                # (exact at v=1; LN variance of ~unit-normal rows is ~1)
                tsl_ = slice(t, t + 1)
                nc.vector.tensor_scalar(vv[:p, tsl_], mv[:p, t, 1:2], EPS,
                                        None, ALU.add)
                nc.vector.reciprocal(y0[:p, tsl_], vv[:p, tsl_])
                nc.vector.tensor_scalar(y0[:p, tsl_], y0[:p, tsl_], 0.5, 0.5,
                                        ALU.mult, ALU.add)
                nc.vector.tensor_tensor(rr[:p, tsl_], vv[:p, tsl_],
                                        y0[:p, tsl_], ALU.mult)
                nc.vector.scalar_tensor_tensor(out=rr[:p, tsl_],
                                               in0=rr[:p, tsl_], scalar=-0.5,
                                               in1=y0[:p, tsl_],
                                               op0=ALU.mult, op1=ALU.mult)
                nc.vector.scalar_tensor_tensor(out=rr[:p, tsl_],
                                               in0=rr[:p, tsl_], scalar=1.5,
                                               in1=y0[:p, tsl_],
                                               op0=ALU.add, op1=ALU.mult)
                nc.vector.tensor_tensor(neg[:p, tsl_], mv[:p, t, 0:1],
                                        rr[:p, tsl_], ALU.mult)
                nc.vector.tensor_scalar(neg[:p, tsl_], neg[:p, tsl_],
                                        -1.0, None, ALU.mult)
                if t % 2 == 0:
                    nc.scalar.activation(core[:p, t], rt, AF.Identity,
                                         bias=neg[:p, tsl_],
                                         scale=rr[:p, tsl_])
                else:
                    nc.vector.tensor_scalar(core[:p, t], rt, rr[:p, tsl_],
                                            neg[:p, tsl_], ALU.mult, ALU.add)
            return core

        core_l = ln_core([raw_l], NQL, "l")
        core = ln_core([raw_f[:, t] for t in range(KC)], N, "f")

        # ---------------- z helpers ----------------

        def z_tile_pe(kc, qb):
            """u8 + ssq matmuls for z tile (kc, qb); shared psum bank."""
            zt = zc_tiles[kc][:, qb].rearrange("c (q k) -> c q k", q=QB)
            zs = zs_tiles[kc][:, qb].rearrange("c (q k) -> c q k", q=QB)
            bank = uzp.tile([128, 512], F32, tag="uz")
            u8 = bank[:, :QB * (H + 1)].rearrange("p (q h) -> p q h", h=H + 1)
            ssq = bank[:, QB * (H + 1):QB * (H + 2)]
            for j in range(QB):
                nc.tensor.matmul(u8[:, j], zt[:, j], w2_sb,
                                 start=(j == 0), stop=False)
            for j in range(QB):
                nc.tensor.matmul(ssq[:, j:j + 1], zs[:, j], ones_col,
                                 start=False, stop=(j == QB - 1))
            return u8, ssq

        def z_pair_stats(pair, u_kc):
            """LN stats for a pair of tiles (both psum banks live). rstd via
            a table-free DVE Newton step: y0 = 0.5 + 0.5/v (exact at v=1),
            rstd ~= y0*(1.5 - 0.5*v*y0^2). u_kc is [k, h, q] so the PE's
            bias-accumulate matmuls read contiguous per-head columns."""
            P2 = QB * len(pair)
            q0 = (pair[0][0] % NQB) * QB
            varb = statp.tile([128, P2], F32, tag="varb")
            for i, (ti, u8, ssq) in enumerate(pair):
                mu2 = statp.tile([128, QB], F32, tag="mu2")
                nc.scalar.square(mu2, u8[:, :, H])
                nc.vector.scalar_tensor_tensor(
                    out=varb[:, i * QB:(i + 1) * QB], in0=ssq,
                    scalar=1.0 / ZD, in1=mu2, op0=ALU.mult, op1=ALU.subtract)
            y0 = statp.tile([128, P2], F32, tag="y0")
            nc.vector.reciprocal(y0, varb)
            nc.vector.tensor_scalar(y0, y0, 0.5, 0.5, ALU.mult, ALU.add)
            t1 = statp.tile([128, P2], F32, tag="t1")
            nc.vector.tensor_tensor(t1, varb, y0, ALU.mult)
            nc.vector.scalar_tensor_tensor(out=t1, in0=t1, scalar=-0.5,
                                           in1=y0, op0=ALU.mult, op1=ALU.mult)
            nc.vector.scalar_tensor_tensor(out=t1, in0=t1, scalar=1.5,
                                           in1=y0, op0=ALU.add, op1=ALU.mult)
            for i, (ti, u8, ssq) in enumerate(pair):
                qs = slice(q0 + i * QB, q0 + (i + 1) * QB)
                nc.vector.tensor_tensor(
                    u_kc[:, :, qs].rearrange("k h q -> k q h"), u8[:, :, :H],
                    t1[:, i * QB:(i + 1) * QB, None].to_broadcast(
                        [128, QB, H]), ALU.mult)

        def emit_zpair(kc, half, u_kc):
            pair = []
            for qb in (2 * half, 2 * half + 1):
                u8, ssq = z_tile_pe(kc, qb)
                pair.append((kc * NQB + qb, u8, ssq))
            z_pair_stats(pair, u_kc)

        def emit_zproj(kc):
            u_kc = ukcp.tile([128, H, NQL], BF16, tag="ukc")
            for half in range(2):
                emit_zpair(kc, half, u_kc)
            return u_kc


        def kproj_kt(t, act_copy=False):
            tsl = slice(t * 128, (t + 1) * 128)
            k_sb = apool.tile([128, D], BF16, tag="knat")
            for sl in _halves(D):
                nn_ = sl.stop - sl.start
                ps = aps.tile([128, 512], F32, tag="proj")
                for c in range(DC):
                    nc.tensor.matmul(ps[:, :nn_], snT[:, c, tsl],
                                     kw_sb[:, c, sl], start=(c == 0),
                                     stop=(c == DC - 1))
                nc.scalar.copy(k_sb[:, sl], ps[:, :nn_])  # ACT: psum evac
            for hc in range(2):
                kt_ps = uzp.tile([48, 8, 128], BF16, tag="uz", name="kt_ps")
                for hh in range(8):
                    h = hc * 8 + hh
                    nc.tensor.transpose(kt_ps[:, hh],
                                        k_sb[:, h * HD:(h + 1) * HD], ident)
                if act_copy:
                    nc.scalar.copy(kT_sb[:, hc * 8:(hc + 1) * 8, tsl], kt_ps)
                else:
                    nc.vector.tensor_copy(kT_sb[:, hc * 8:(hc + 1) * 8, tsl],
                                          kt_ps)

        def vproj(t):
            tsl = slice(t * 128, (t + 1) * 128)
            for sl in (slice(0, 480), slice(480, 768)):
                nn_ = sl.stop - sl.start
                h0 = sl.start // HD
                nh = nn_ // HD
                ps = aps.tile([128, 512], F32, tag="proj")
                for c in range(DC):
                    nc.tensor.matmul(ps[:, :nn_], snT[:, c, tsl],
                                     vw_sb[:, c, sl], start=(c == 0),
                                     stop=(c == DC - 1))
                nc.scalar.copy(v_sb[:, t, h0:h0 + nh, :HD],
                               ps[:, :nn_].rearrange("p (h d) -> p h d", d=HD))

        # psum pools for the lead-in + loop: uzp(2) + sps(2) + aps(2) +
        # apt1(2) = 8 banks; apt1 closes before ops_ (o accumulator) opens,
        # and the tail reuses the aps ring, keeping the pool trace stack-LIFO.
        sps = top.enter_context(tc.tile_pool(name="sps", bufs=2, space="PSUM"))
        aps = top.enter_context(tc.tile_pool(name="aps", bufs=2, space="PSUM"))

        # snT: transpose LN'd s to [d, tok], per-(d,t) 128x128 blocks so the
        # k pipeline can consume token-chunk 0 as soon as it exists
        snT = apool.tile([128, DC, N], BF16)
        snT_loc = apool.tile([128, DC, NQL], BF16)
        with tc.tile_pool(name="apt1", bufs=1, space="PSUM") as apt1:
            ps_l = apt1.tile([128, DC, NQL], BF16, tag="tpl",
                             padded_shape=[128, KC, 128])
            for d in range(DC):
                nc.tensor.transpose(ps_l[:, d],
                                    core_l[:NQL, 0, d * 128:(d + 1) * 128],
                                    ident[:NQL, :NQL])
            nc.vector.tensor_copy(snT_loc, ps_l)
            tpd = apt1.tile([128, DC, 128], BF16, tag="tpd")
            ci = 0
            for t in range(KC):
                tsl = slice(t * 128, (t + 1) * 128)
                for d in range(DC):
                    nc.tensor.transpose(tpd[:, d],
                                        core[:, t, d * 128:(d + 1) * 128],
                                        ident)
                    if ci % 2 == 0:
                        nc.scalar.copy(snT[:, d, tsl], tpd[:, d])
                    else:
                        nc.vector.tensor_copy(snT[:, d, tsl], tpd[:, d])
                    ci += 1
                if t == 0:
                    # q local projection + qT fill the wait for zc0's DMA
                    q_sb = apool.tile([NQL, D], BF16, tag="qnat")
                    for sl in _halves(D):
                        nn_ = sl.stop - sl.start
                        ps = aps.tile([128, 512], F32, tag="proj")
                        nc.tensor.matmul(ps[:NQL, :nn_], ones_row[0:1, :NQL],
                                         qb_sb[:, sl], start=True, stop=False)
                        for c in range(DC):
                            nc.tensor.matmul(ps[:NQL, :nn_], snT_loc[:, c],
                                             qw_sb[:, c, sl], start=False,
                                             stop=(c == DC - 1))
                        nc.vector.tensor_copy(q_sb[:, sl], ps[:NQL, :nn_])
                    for hc in range(2):
                        qt_ps = uzp.tile([48, 8, 128], BF16, tag="uz",
                                         name="qt_ps")
                        for hh in range(8):
                            h = hc * 8 + hh
                            nc.tensor.transpose(qt_ps[:, hh, :NQL],
                                                q_sb[:, h * HD:(h + 1) * HD],
                                                ident[:NQL, :NQL])
                        nc.vector.tensor_copy(qT_sb[:, hc * 8:(hc + 1) * 8],
                                              qt_ps[:, :, :NQL])
                    u_kcs = {0: emit_zproj(0)}
                    kproj_kt(0)
                if t == 1:
                    kproj_kt(1)

        raw_st.close()
        ops_ = top.enter_context(tc.tile_pool(name="ops", bufs=1, space="PSUM"))
        o_ps = ops_.tile([NQL, H, 64], F32)
        # o_ps starts from vb (broadcast over queries): softmax rows sum to 1,
        # so adding vb once up front equals adding it to every v row.
        vbp_a = qw_sb[32:33, DC, 0:512]
        vbp_b = qw_sb[64:65, DC, 0:512]
        nc.tensor.matmul(o_ps[:, 0:8], ones_row[32:33, :NQL], vbp_a,
                         start=True, stop=False)
        nc.tensor.matmul(o_ps[:, 8:16], ones_row[64:65, :NQL], vbp_b,
                         start=True, stop=False)

        def scores_exp_kc(kc, u_kc):
            """scores + pair-bias accumulate on PE, exp from PSUM on ACT."""
            ksl = slice(kc * 128, (kc + 1) * 128)
            p_sb = kcp.tile([128, H, NQL], BF16, tag="psb")
            for hg in range(NHG):
                h0 = hg * HG
                s_ps = sps.tile([128, 512], F32, tag="sps")
                s4 = s_ps[:, :HG * NQL].rearrange("p (h q) -> p h q", q=NQL)
                for hh in range(HG):
                    h = h0 + hh
                    nc.tensor.matmul(s4[:, hh], kT_sb[:, h, ksl], qT_sb[:, h],
                                     start=(hh == 0), stop=False)
                for hh in range(HG):
                    h = h0 + hh
                    nc.tensor.matmul(s4[:, hh], ident, u_kc[:, h],
                                     start=False, stop=(hh == HG - 1))
                nc.scalar.activation(p_sb[:, h0:h0 + HG], s4, AF.Exp,
                                     bias=shift_sb)
            return p_sb

        def pv_kc(kc, p_sb):
            for h in range(H):
                nc.tensor.matmul(o_ps[:, h, :HD + 1], p_sb[:, h],
                                 v_sb[:, kc, h, :],
                                 start=False,
                                 stop=(kc == KC - 1 and h % 8 == 7))

        # -------- main pipelined loop over k-chunks --------
        # k projection runs two chunks ahead so kT(t) is already in SBUF when
        # scores(t) issue; the per-chunk critical chain is then just
        # u_kc(t) -> scores(t) -> exp(t) -> PV(t).
        pend = {}
        for t in range(KC):
            # first half of the next chunk's z projection goes ahead of the
            # scores so its DVE stats overlap this chunk's exp/PV
            if t + 1 < KC:
                u_next = ukcp.tile([128, H, NQL], BF16, tag="ukc",
                                   name="u_next")
                emit_zpair(t + 1, 0, u_next)
            pend[t] = scores_exp_kc(t, u_kcs.pop(t))
            if t >= 1:
                pv_kc(t - 1, pend.pop(t - 1))
            if t + 1 < KC:
                emit_zpair(t + 1, 1, u_next)
                u_kcs[t + 1] = u_next
            if t + 2 < KC:
                kproj_kt(t + 2)
            if t == 4:
                ow_sb = persist.tile([128, DC, D], BF16)
                nc.gpsimd.dma_start(ow_sb,
                                    ow.ap().rearrange("(c p) m -> p c m",
                                                      p=128))
            vproj(t)
        pv_kc(KC - 1, pend.pop(KC - 1))

        # g projection (off the critical path; gw arrived long ago)
        for sl in _halves(D):
            nn_ = sl.stop - sl.start
            ps = aps.tile([128, 512], F32, tag="proj")
            nc.tensor.matmul(ps[:NQL, :nn_], ones_row[:, :NQL],
                             gb_sb[:, sl], start=True, stop=False)
            for c in range(DC):
                nc.tensor.matmul(ps[:NQL, :nn_], snT_loc[:, c],
                                 gw_sb[:, c, sl], start=False,
                                 stop=(c == DC - 1))
            nc.scalar.activation(g_sb[:, sl], ps[:NQL, :nn_], AF.Exp,
                                 scale=-1.0)
        # g = sigmoid(x) = 1 / (1 + exp(-x)), finished on DVE
        with nc.allow_low_precision(reason="bf16 gate: 1+exp(-x) then recip"):
            nc.vector.tensor_scalar(g_sb, g_sb, 1.0, None, ALU.add)
            nc.vector.reciprocal(g_sb, g_sb)

        # ================= tail: normalize, gate, project =================
        pa_sb.close()
        tail = top.enter_context(tc.tile_pool(name="tail", bufs=1))
        rcp = tail.tile([NQL, H], F32)
        o_sb = tail.tile([NQL, H, HD], BF16)
        og = tail.tile([NQL, D], BF16)
        ogt = tail.tile([128, DC, NQL], BF16)
        out_ps = [aps.tile([NQL, 512], F32, tag="proj", name=f"out_ps{i}")
                  for i in range(2)]
        out_sb = [tail.tile([NQL, 512], BF16, name=f"out_sb{i}")
                  for i in range(2)]
        # head-half pipelining: o_ps bank 0 (heads 0-7) finishes PV first, so
        # its normalize/gate/transpose overlaps the second bank's PV drain,
        # and the out projection accumulates c-chunks as they appear
        for hb in range(2):
            hs = slice(hb * 8, hb * 8 + 8)
            nc.vector.reciprocal(rcp[:, hs], o_ps[:, hs, HD])
            nc.vector.tensor_tensor(o_sb[:, hs], o_ps[:, hs, :HD],
                                    rcp[:, hs, None].to_broadcast(
                                        [NQL, 8, HD]), ALU.mult)
            csl = slice(hb * 384, hb * 384 + 384)
            nc.gpsimd.tensor_mul(og[:, csl],
                                  o_sb.rearrange("q h d -> q (h d)")[:, csl],
                                  g_sb[:, csl])
            ogt_ps = uzp.tile([128, 3, NQL], BF16, tag="uz",
                              name=f"ogt_ps{hb}")
            for dd in range(3):
                d = hb * 3 + dd
                nc.tensor.transpose(ogt_ps[:, dd],
                                    og[:, d * 128:(d + 1) * 128],
                                    ident[:NQL, :NQL])
            nc.vector.tensor_copy(ogt[:, hb * 3:hb * 3 + 3], ogt_ps)
            for i, sl in enumerate(_halves(D)):
                nn_ = sl.stop - sl.start
                for dd in range(3):
                    c = hb * 3 + dd
                    nc.tensor.matmul(out_ps[i][:, :nn_], ogt[:, c],
                                     ow_sb[:, c, sl], start=(c == 0),
                                     stop=(c == DC - 1))
        for i, sl in enumerate(_halves(D)):
            nn_ = sl.stop - sl.start
            if i == 0:
                nc.scalar.copy(out_sb[i][:, :nn_], out_ps[i][:, :nn_])
            else:
                nc.vector.tensor_copy(out_sb[i][:, :nn_], out_ps[i][:, :nn_])
            eng = nc.sync if i == 0 else nc.scalar
            eng.dma_start(out.ap()[:, sl], out_sb[i][:, :nn_])

    _split_excess_waits(nc)
    return nc


def prep_inputs(inputs, N=768, D=768, H=16, HD=48, ZD=128, n_cores=N_CORES,
                QB=24):
    """Host-side: fold LN(s) weights + scale into projections; cast z to
    bf16 / z^2 to fp8 and transpose per core to [kc, qb, c, q, k]."""
    f32 = np.float32
    bf16 = ml_dtypes.bfloat16
    fp8 = ml_dtypes.float8_e4m3
    s = np.asarray(inputs["s"], f32).reshape(N, D)
    z = np.asarray(inputs["z"], f32).reshape(N, N, ZD)
    wv = np.asarray(inputs["norm_s_w"], f32)
    bv = np.asarray(inputs["norm_s_b"], f32)
    scale = HD ** -0.5

    def fold(wm, bias_extra=None, sc=1.0):
        wm = np.asarray(wm, f32)
        wf = (wv[:, None] * wm) * sc
        bf = (bv @ wm) * sc
        if bias_extra is not None:
            bf = bf + np.asarray(bias_extra, f32) * sc
        return wf, bf

    qwf, qbf = fold(inputs["q_w"], inputs["q_b"], scale)
    kwf, _ = fold(inputs["k_w"])          # k bias is softmax-invariant: drop
    vwf, vbf = fold(inputs["v_w"])
    gwf, gbf = fold(inputs["g_w"])
    W2 = (np.asarray(inputs["z_norm_w"], f32)[:, None]
          * np.asarray(inputs["z_w"], f32))
    alpha = W2.sum(axis=0)
    W2aug = np.concatenate(
        [W2 - alpha[None, :] / ZD, np.full((ZD, 1), 1.0 / ZD, f32)], axis=1)

    zb = z.astype(fp8)
    zsqb = (z * z).astype(fp8)
    sb = s.astype(bf16)
    NQL = N // n_cores
    KC = N // 128
    NQB = NQL // QB

    def with_bias(wf, bf, extra=None):
        ext = np.zeros((128, D), f32)
        ext[0] = bf
        if extra is not None:
            ext[32] = extra[:D]
            ext[64, :len(extra) - D] = extra[D:]
        return np.concatenate([wf, ext]).astype(bf16)

    vb_pad = np.zeros((16, 64), f32)
    vb_pad[:, :HD] = vbf.reshape(H, HD)
    shared = {
        "s_full": sb,
        "qw": with_bias(qwf, qbf, vb_pad.ravel()), "kw": kwf.astype(bf16),
        "vw": vwf.astype(bf16), "gw": with_bias(gwf, gbf),
        "ow": np.asarray(inputs["o_w"], f32).astype(bf16),
        "w2": W2aug.astype(bf16),
    }

    def zshape(a):
        zt = a.reshape(NQB, QB, KC, 128, ZD)                 # qb qq kc kk c
        zt = np.ascontiguousarray(zt.transpose(2, 0, 4, 1, 3))
        return zt.reshape(KC, NQB, ZD, QB * 128)

    in_maps = []
    for d in range(n_cores):
        m = dict(shared)
        m["s_loc"] = np.ascontiguousarray(sb[d * NQL:(d + 1) * NQL])
        m["zq"] = zshape(zb[d * NQL:(d + 1) * NQL])
        m["zsq"] = zshape(zsqb[d * NQL:(d + 1) * NQL])
        in_maps.append(m)
    return in_maps


_CACHED = {}


def kernel(**inputs) -> np.ndarray:
    from concourse.bass_utils import run_bass_kernel_spmd
    N, D = 768, 768
    if "nc" not in _CACHED:
        _CACHED["nc"] = build_kernel()
    nc = _CACHED["nc"]
    in_maps = prep_inputs(inputs)
    res = run_bass_kernel_spmd(nc, in_maps, core_ids=list(range(N_CORES)))
    _CACHED["last_result"] = res
    out = np.concatenate([np.asarray(r["out"], dtype=np.float32)
                          for r in res.results], axis=0)
    return out.reshape(1, N, D)
